# revision 15
# baseline (speedup 1.0000x reference)
"""DeepseekMoE block-quantized MoE kernel for 8 Trainium2 NeuronCores.

Strategy (expert-parallel with host-side dispatch):
  - The routing table (selected_experts) is known on the host before launch,
    so the all-to-all "dispatch" is done on the host: for each expert e we
    gather the unique tokens routed to it (dedup across the top-k slots),
    transpose to [H, n_e], and pad to a common capacity C.
  - Experts are sharded 2-per-core across the 8 cores.  Each core runs a
    dense 3-matmul MLP (gate/up -> silu*up -> down) for its 2 experts in
    x^T / act^T layout so no on-device transposes are needed.
  - Block-dequantization (w * repeat(s, 128)) is folded into the host-side
    weight preparation, which also pre-permutes the weights into slab-major
    order (gate and up packed in one tensor) so every weight DMA is one
    contiguous transfer per partition.
  - All matmul operands are fp16 (1 col/cycle on the PE like bf16, but with
    10 mantissa bits): halves HBM traffic, SBUF footprint, and SBUF->PE
    stream bandwidth vs fp32r while staying well inside the 2e-2 tolerance.
  - The sync (SP) sequencer issues DMA descriptors at only ~1.6/us, so DMA
    count is minimized (one dma_start per x chunk covering all 16 h-tiles,
    one per weight slab pair) and y stores are issued from the scalar
    (Activation) HWDGE queue to keep the sync queue off the critical path.
  - Startup: a short burst of dependency-free warmup matmuls ramps the PE
    HAM clock gate from 1.2 to 2.4 GHz during the DMA pre-roll; the first
    real weight slab is split in 4 so the first matmul gates on ~128KB.
  - The host scatters the per-expert outputs back to [T, K, H].
"""

import math

import numpy as np

T = 4096
TOPK = 6
E = 16
H = 2048
I = 1408
BS = 128           # quant block size
HT = H // 128      # 16 h-tiles
IT = I // 128      # 11 i-tiles
NCORES = 8
# SBUF bound: both jobs' x (64W B/partition) + shared acts (22W) + ~65KB
# of weight/output staging must fit in ~208KB usable per partition.
MAX_W = 1550
NWARM = 24         # PE warmup matmuls: ~3.4us cold ramp + keep-warm until data

_BUILT = {}
LAST_RESULTS = None  # stashed BassKernelResults for external harnesses


def _chunk_plan(width, cmax=512):
    """Split `width` columns into chunks of at most `cmax`."""
    if width <= cmax:
        return [(0, width)]
    n = -(-width // cmax)
    base = (width // n) // 8 * 8
    rem8 = (width - n * base) // 8
    out, off = [], 0
    for j in range(n):
        w = base + (8 if j < rem8 else 0)
        if j == n - 1:
            w = width - off
        out.append((off, w))
        off += w
    return out


def _build(jobs, CT):
    """Build the SPMD Bass program.  `jobs` is a tuple of
    (slot, col_offset, width): each job runs one expert slot's MLP over a
    window of `width` token columns; CT is the column capacity of xt/yt."""
    import concourse.bacc as bacc
    import concourse.mybir as mybir
    from concourse.bass import ts
    from concourse.tile import TileContext

    f32 = mybir.dt.float32
    f16 = mybir.dt.float16
    AF = mybir.ActivationFunctionType
    import os as _os

    act_fn = (
        AF.Sigmoid if _os.environ.get("KERNEL_SIM_SIGMOID") else AF.Silu
    )  # CoreSim lacks Silu; HW path always uses Silu

    nc = bacc.Bacc()
    # Slab-major layouts (host pre-permuted):
    #   xt[s, h, p, c]      = x[col c of slot s][h*128+p]
    #   w01t[s, i, p, hj]   = W0^T[h*128+p][i*128+j]          (hj = h*128+j)
    #   w01t[s, i, p, H+hj]  = W1^T[h*128+p][i*128+j]
    #   w2t[s, h, p, ij]    = W2^T[i*128+p][h*128+j]          (ij = i*128+j)
    xt = nc.declare_dram_parameter("xt", [2, HT, 128, CT], f16, isOutput=False)
    w01t = nc.declare_dram_parameter("w01t", [2, IT, 128, 2 * H], f16, isOutput=False)
    w2t = nc.declare_dram_parameter("w2t", [2, HT, 128, I], f16, isOutput=False)
    yt = nc.declare_dram_parameter("yt", [2, HT, 128, CT], f32, isOutput=True)

    with TileContext(nc) as tc:
        with (
            tc.tile_pool(name="xp", bufs=1) as xp,
            tc.tile_pool(name="ap", bufs=1) as apool,
            tc.tile_pool(name="wp", bufs=2) as wp,
            tc.tile_pool(name="yp", bufs=2) as yp,
            tc.tile_pool(name="ps", bufs=2, space="PSUM") as ps,
        ):
            # ---- PE warmup: dependency-free matmuls on a memset scratch
            # tile ramp the HAM clock gate to 2.4 GHz while the DMA rings
            # spin up and the first real operands land.
            warm_sb = wp.tile([128, 512], f16, tag="warm", bufs=1)
            nc.vector.memset(warm_sb, 0.0)
            warm_ps = ps.tile([128, 512], f32, tag="warm", bufs=1)
            for _ in range(NWARM):
                nc.tensor.matmul(
                    warm_ps, warm_sb[:, 0:128], warm_sb, start=True, stop=True
                )

            for jn, (s, co, W) in enumerate(jobs):
                    # 512-col chunks: one PSUM bank per matmul output.
                    chunks_a = _chunk_plan(W, 512)
                    chunks_b = _chunk_plan(W, 512)
                    split_first = jn == 0

                    # First gate/up slab for this job.  Job 0's gate half is
                    # split in 4 so the very first matmul gates on 1/4 of the
                    # slab bytes; later jobs load the pair in one transfer.
                    if split_first:
                        w0p = []
                        for q in range(4):
                            tile = wp.tile([128, 512], f16, tag=f"w0p{q}", bufs=1)
                            nc.sync.dma_start(
                                out=tile, in_=w01t[s, 0, :, q * 512 : (q + 1) * 512]
                            )
                            w0p.append(tile)
                    else:
                        w01f = wp.tile([128, 2 * H], f16, tag="w01f", bufs=1)
                        nc.sync.dma_start(out=w01f, in_=w01t[s, 0])

                    # x for this job: one dma_start per chunk covering all 16
                    # h-tiles (xc[p, h*cw + c] = xt[s, h, p, co+c0+c]).  Job
                    # 0's first chunk is split into two h-halves so the first
                    # gate matmuls start as soon as half the bytes land.
                    xparts = []  # per chunk: list of (h_lo, h_n, tile)
                    for ci, (c0, cw) in enumerate(chunks_a):
                        hsplits = (
                            [(0, 8), (8, 8)] if (split_first and ci == 0) else [(0, HT)]
                        )
                        parts = []
                        for pi, (h_lo, h_n) in enumerate(hsplits):
                            xc = xp.tile(
                                [128, h_n * cw], f16,
                                tag=f"xc{ci}_{pi}_{s}", name=f"xc{ci}_{pi}_{jn}",
                            )
                            nc.sync.dma_start(
                                out=xc.rearrange("p (h c) -> p h c", h=h_n),
                                in_=xt[
                                    s, h_lo : h_lo + h_n, :, co + c0 : co + c0 + cw
                                ].rearrange("h p c -> p h c"),
                            )
                            parts.append((h_lo, h_n, xc))
                        xparts.append(parts)
                        if split_first and ci == 0:
                            # up-proj half of job0's first slab
                            w1f = wp.tile([128, H], f16, tag="w1f", bufs=1)
                            nc.sync.dma_start(out=w1f, in_=w01t[s, 0, :, H : 2 * H])

                    def xop(ci, h, cw):
                        for h_lo, h_n, xc in xparts[ci]:
                            if h_lo <= h < h_lo + h_n:
                                return xc[:, (h - h_lo) * cw : (h - h_lo) * cw + cw]
                        raise AssertionError

                    acts = [
                        apool.tile([128, W], f16, tag=f"a{i}", name=f"a{i}_{jn}")
                        for i in range(IT)
                    ]

                    # Phase A: gate/up projections + silu*up, per i-tile.
                    for i in range(IT):
                        if i == 0:
                            w01s = None if split_first else w01f
                        else:
                            w01s = wp.tile([128, 2 * H], f16, tag="w01", name=None)
                            nc.sync.dma_start(out=w01s, in_=w01t[s, i])
                        for ci, (c0, cw) in enumerate(chunks_a):
                            g = ps.tile([128, 512], f32, tag="g")
                            for h in range(HT):
                                if i == 0 and split_first:
                                    stat = w0p[h // 4][:, ts(h % 4, 128)]
                                elif i == 0:
                                    stat = w01f[:, ts(h, 128)]
                                else:
                                    stat = w01s[:, ts(h, 128)]
                                nc.tensor.matmul(
                                    g[:, :cw],
                                    stat,
                                    xop(ci, h, cw),
                                    start=(h == 0),
                                    stop=(h == HT - 1),
                                )
                            u = ps.tile([128, 512], f32, tag="u")
                            for h in range(HT):
                                if i == 0 and split_first:
                                    stat = w1f[:, ts(h, 128)]
                                elif i == 0:
                                    stat = w01f[:, H + h * 128 : H + (h + 1) * 128]
                                else:
                                    stat = w01s[:, H + h * 128 : H + (h + 1) * 128]
                                nc.tensor.matmul(
                                    u[:, :cw],
                                    stat,
                                    xop(ci, h, cw),
                                    start=(h == 0),
                                    stop=(h == HT - 1),
                                )
                            a_sl = acts[i][:, c0 : c0 + cw]
                            nc.scalar.activation(a_sl, g[:, :cw], act_fn)
                            nc.vector.tensor_mul(a_sl, a_sl, u[:, :cw])

                    # Phase B: down projection, per h-tile.  Chunk outputs
                    # collect into one [128, W] tile and store in one DMA
                    # (the sync queue has descriptor slack now that loads are
                    # merged).  The last h-tile stores per chunk to shorten
                    # the drain after the final matmul.
                    for h in range(HT):
                        w2s = wp.tile([128, I], f16, tag=f"w2_{s}", bufs=2)
                        nc.sync.dma_start(out=w2s, in_=w2t[s, h])
                        last_h = h == HT - 1
                        yc = yp.tile([128, W], f32, tag="y")
                        for c0, cw in chunks_b:
                            o = ps.tile([128, 512], f32, tag="o")
                            for i in range(IT):
                                nc.tensor.matmul(
                                    o[:, :cw],
                                    w2s[:, ts(i, 128)],
                                    acts[i][:, c0 : c0 + cw],
                                    start=(i == 0),
                                    stop=(i == IT - 1),
                                )
                            nc.vector.tensor_copy(yc[:, c0 : c0 + cw], o[:, :cw])
                            if last_h:
                                nc.sync.dma_start(
                                    out=yt[s, h, :, co + c0 : co + c0 + cw],
                                    in_=yc[:, c0 : c0 + cw],
                                )
                        if not last_h:
                            nc.sync.dma_start(
                                out=yt[s, h, :, co : co + W], in_=yc
                            )
    nc.finalize()
    return nc


def _get_built(jobs, CT):
    key = (tuple(jobs), CT)
    if key not in _BUILT:
        _BUILT[key] = _build(tuple(jobs), CT)
    return _BUILT[key]


def _dequant(w, s):
    """w: [E, O, Iin], s: [E, O, Iin//128] -> dequantized [E, O, Iin] fp32."""
    e, o, iin = w.shape
    return (w.reshape(e, o, iin // BS, BS) * s[..., None]).reshape(e, o, iin)


def _slab_major(wd):
    """wd: [E, O, Iin] dequantized.  Returns [E, O//128, 128, Iin] where
    out[e, oc, p, b*128+j] = wd[e][oc*128+j][b*128+p]: the slab for output
    block oc holds, per contraction block b, a [128(contraction-sub p) x
    128(output-sub j)] stationary tile, contiguous along the free dim."""
    e, o, iin = wd.shape
    return np.ascontiguousarray(
        wd.reshape(e, o // 128, 128, iin // 128, 128).transpose(0, 1, 4, 3, 2)
    ).reshape(e, o // 128, 128, iin)


def kernel(**inputs):
    global LAST_RESULTS
    x = np.ascontiguousarray(np.asarray(inputs["x"], dtype=np.float32))
    sel = np.asarray(inputs["selected_experts"])
    w0 = np.asarray(inputs["w0"], dtype=np.float32)
    s0 = np.asarray(inputs["s0"], dtype=np.float32)
    w1 = np.asarray(inputs["w1"], dtype=np.float32)
    s1 = np.asarray(inputs["s1"], dtype=np.float32)
    w2 = np.asarray(inputs["w2"], dtype=np.float32)
    s2 = np.asarray(inputs["s2"], dtype=np.float32)

    t, k = sel.shape
    assert (t, k) == (T, TOPK) and x.shape == (T, H)

    # ---- host-side dispatch: unique tokens per expert ----
    pos = np.full((E, T), -1, dtype=np.int32)
    cols = []
    for e in range(E):
        toks = np.nonzero((sel == e).any(axis=1))[0]
        cols.append(toks)
        pos[e, toks] = np.arange(len(toks), dtype=np.int32)
    counts = np.array([len(c) for c in cols])

    # Assign experts to (core, slot): slot 0 holds the 8 largest experts,
    # slot 1 the 8 smallest, so each slot's padded width is only the max of
    # its own rank group.  expert_of[s][c] = expert on core c, slot s.
    order = np.argsort(-counts, kind="stable")
    expert_of = [list(order[:NCORES]), list(order[NCORES:])]

    def align8(v):
        return max(256, -(-v // 8) * 8)

    slot_w = [align8(int(counts[expert_of[s]].max())) for s in range(2)]

    if max(slot_w) <= MAX_W:
        jobs = tuple((s, 0, slot_w[s]) for s in range(2))
        CT = max(slot_w)
    else:
        # fallback: uniform width, multiple column windows per slot
        cmax = int(counts.max())
        passes = max(1, math.ceil(cmax / MAX_W))
        W = align8(math.ceil(cmax / passes))
        CT = W * passes
        jobs = tuple((s, cp * W, W) for s in range(2) for cp in range(passes))

    # ---- dequantize + slab-major permute weights (host), cast fp16 ----
    w0t_all = _slab_major(_dequant(w0, s0))  # [E, IT, 128, H]
    w1t_all = _slab_major(_dequant(w1, s1))  # [E, IT, 128, H]
    w01t_all = np.concatenate([w0t_all, w1t_all], axis=-1)  # [E, IT, 128, 2H]
    w2t_all = _slab_major(_dequant(w2, s2))  # [E, HT, 128, I]

    xT16 = x.T.astype(np.float16)  # [H, T]

    in_maps = []
    for c in range(NCORES):
        pair = [expert_of[0][c], expert_of[1][c]]
        xt_c = np.zeros((2, H, CT), dtype=np.float16)
        for s, e in enumerate(pair):
            n = len(cols[e])
            if n:
                xt_c[s, :, :n] = xT16[:, cols[e]]
        in_maps.append(
            {
                "xt": xt_c.reshape(2, HT, 128, CT),
                "w01t": w01t_all[pair].astype(np.float16),
                "w2t": w2t_all[pair].astype(np.float16),
            }
        )

    nc = _get_built(jobs, CT)
    from concourse.bass_utils import run_bass_kernel_spmd

    res = run_bass_kernel_spmd(nc, in_maps, list(range(NCORES)))
    LAST_RESULTS = res

    # Y[e] = [H, CT] for expert e
    Y = np.empty((E, H, CT), dtype=np.float32)
    for c in range(NCORES):
        yt_c = np.asarray(res.results[c]["yt"]).reshape(2, H, CT)
        Y[expert_of[0][c]] = yt_c[0]
        Y[expert_of[1][c]] = yt_c[1]

    # ---- scatter back to [T, K, H] ----
    e_flat = sel.reshape(-1).astype(np.int64)
    t_flat = np.repeat(np.arange(T, dtype=np.int64), TOPK)
    p_flat = pos[e_flat, t_flat]
    out = Y[e_flat, :, p_flat]  # [T*K, H]
    return np.ascontiguousarray(out.reshape(T, TOPK, H), dtype=np.float32)
